# revision 25
# baseline (speedup 1.0000x reference)
"""Multi-head causal self-attention (RoPE) on 8 TRN2 NeuronCores.

Strategy (tensor-parallel over heads, per the sharding hint):
  - 16 heads / 8 cores -> 2 heads per core. Each core processes ALL 4
    batches for its 2 heads:
      qkv slice -> RoPE -> causal softmax(q k^T) v -> partial out-proj
    and writes a full-shape partial y (row-parallel w_proj) in fp16. The
    host sums the 8 partials in fp32 and adds b_proj.
  - All matmul operands are fp16 (PSUM accumulation stays fp32): the PE
    streams 1 row/cycle at ANY free size (fp32r needs >=256), DVE runs
    elementwise fp16-SBUF ops at 4x, and DMA bytes halve.
  - x is sent pre-transposed (x^T, [C, T] per batch) so the contraction
    dim C lands on SBUF partitions with no on-device transposes.
  - Attention runs in the "S^T" layout (k-tokens on partitions, q-tokens
    on the free dim):
      S^T tile   = matmul(lhsT=k^T[:,ktile], rhs=q^T[:,qchunk])
      P^T        = exp(S^T * 1/sqrt(D))      (ACT, no max-subtraction:
                                              |scores| <~ 6 so exp is safe)
      denom      = matmul(lhsT=ones[128,1], rhs=P^T)   (partition sum)
      out^T      = matmul(lhsT=v[ktile,:], rhs=P^T)    accumulated
      attn_out^T = out^T * partition_broadcast(1/denom)
    attn_out^T ([d, tok]) is directly the lhsT of the out-projection.
  - Causal masking: for the 4 diagonal k-tiles of each 512-wide q chunk
    the S/exp/denom/out matmuls are restricted to the valid q sub-range
    (saves ~15% of attention PE rows); only the 128-wide triangular part
    needs an elementwise mask (one constant [128,128] tri mask, DVE 4x).
  - Stage B is ONE flattened software pipeline over (head, chunk, k-tile
    pair) with chunks processed descending (long chunks first) so the PE
    stream never drains at chunk/head boundaries. S tiles are written in
    pairs into a [128,1024] 2-bank PSUM tile and evicted by a single
    wide exp to halve ACT per-instruction overhead (ACT is the stage-B
    co-bottleneck: every P element passes it at 1 elem/lane/cycle).
  - RoPE: the head dim d sits on partitions; rotate_half needs rows
    d <-> d+-64. We permute the d index on the host (within each head's
    128 columns of w_qkv + the cos/sin tables) so that rotation partners
    sit 16 apart inside the same 32-partition quadrant, which a single
    DVE stream_shuffle implements. Scores are invariant to the (shared)
    q/k permutation.
  - DMA: startup loads are ordered first-needed-first (wqk[kc]+xt[c0,kc]
    interleaved) so the PE starts ~1us in instead of waiting ~33us for
    all weights; x prefetch runs on the sync queue while y writebacks
    use the vector queue so neither blocks the other.
"""

from contextlib import ExitStack

import numpy as np

import concourse.bacc as bacc
import concourse.bass as bass
import concourse.mybir as mybir
import concourse.tile as tile
from concourse.bass import ds

B = 4
T = 2048
C = 2048
H = 16
D = 128
NCORES = 8
HPC = H // NCORES  # heads per core = 2
KC = C // 128  # 16 contraction tiles
TT = T // 128  # 16 token tiles
ACH = 256  # stage-A token chunk
NACH = T // ACH
QCH = 512  # stage-B q chunk
NQCH = T // QCH
LAGP = 3  # stage-B pair pipeline depth (consumers trail producers)
PTBUFS = 8  # pt2 slots: exp(w)'s slot-reuse WAR must point far into the past
            # (a shallow pool couples exp to recent PE blocks via scheduler
            # wait-instructions in the ACT stream, setting a ~1.7us/pair floor)
INV_SQRT_D = float(1.0 / np.sqrt(np.float32(D)))

F32 = mybir.dt.float32
F16 = mybir.dt.float16

# d-permutation: quadrant s holds original d = s*16..s*16+15 (rows 0-15)
# and d+64 partners (rows 16-31); swap = stream_shuffle by +-16.
PERM = np.concatenate(
    [np.concatenate([np.arange(s * 16, s * 16 + 16), 64 + np.arange(s * 16, s * 16 + 16)]) for s in range(4)]
).astype(np.int64)
SWAP_MASK = [(i + 16) % 32 for i in range(32)]


def build_program():
    nc = bacc.Bacc("TRN2", target_bir_lowering=False, debug=False, num_devices=NCORES)

    xt = nc.dram_tensor("xt", [B, KC, 128, T], F16, kind="ExternalInput").ap()
    # weights packed so every DMA moves >=2KB per partition line (the DMA
    # engines are descriptor-feed-bound: bytes/line is the throughput lever)
    wqk = nc.dram_tensor("wqk", [KC // 2, 128, 1024], F16, kind="ExternalInput").ap()
    wv = nc.dram_tensor("wv", [KC // 4, 128, 1024], F16, kind="ExternalInput").ap()
    wproj = nc.dram_tensor("wproj", [HPC, 128, C], F16, kind="ExternalInput").ap()
    cos_in = nc.dram_tensor("cos_t", [128, T], F16, kind="ExternalInput").ap()
    sin_in = nc.dram_tensor("sin_t", [128, T], F16, kind="ExternalInput").ap()
    tri_in = nc.dram_tensor("tri", [128, 128], F16, kind="ExternalInput").ap()
    ones_in = nc.dram_tensor("ones", [128, 1], F16, kind="ExternalInput").ap()
    y = nc.dram_tensor("y", [B, TT, 128, C], F16, kind="ExternalOutput").ap()

    with TileKernel(nc) as tk:
        tk.build(xt, wqk, wv, wproj, cos_in, sin_in, tri_in, ones_in, y)
    nc.compile()
    return nc


class TileKernel:
    def __init__(self, nc):
        self.nc = nc
        self.stack = ExitStack()

    def __enter__(self):
        self.tc = self.stack.enter_context(tile.TileContext(self.nc))
        return self

    def __exit__(self, *exc):
        return self.stack.__exit__(*exc)

    def build(self, xt, wqk, wv, wproj, cos_in, sin_in, tri_in, ones_in, y):
        nc, tc = self.nc, self.tc
        ctx = self.stack
        from concourse import library_config
        nc.gpsimd.load_library(library_config.attn)

        consts = ctx.enter_context(tc.tile_pool(name="consts", bufs=1))
        store = ctx.enter_context(tc.tile_pool(name="store", bufs=1))
        xtp = ctx.enter_context(tc.tile_pool(name="xtp", bufs=40))
        qkfp = ctx.enter_context(tc.tile_pool(name="qkfp", bufs=4))
        ropep = ctx.enter_context(tc.tile_pool(name="ropep", bufs=4))
        pp = ctx.enter_context(tc.tile_pool(name="pp", bufs=PTBUFS))
        rp = ctx.enter_context(tc.tile_pool(name="rp", bufs=3))
        evp = ctx.enter_context(tc.tile_pool(name="evp", bufs=8))

        # ---- persistent weights / tables ----
        self.wqk_sb = consts.tile([128, KC, 512], F16, name="wqk_sb")
        self.wv_sb = consts.tile([128, KC, 256], F16, name="wv_sb")
        self.wproj_sb = consts.tile([128, HPC, C], F16, name="wproj_sb")
        self.cos_sb = consts.tile([128, T], F16, name="cos_sb")
        self.sin_sb = consts.tile([128, T], F16, name="sin_sb")
        self.tri_sb = consts.tile([128, 128], F16, name="tri_sb")
        self.ones_col = consts.tile([128, 1], F16, name="ones_col")

        # ---- per-batch stores (layouts: [d, tok] except v = [tok, d]) ----
        self.q_t = [store.tile([128, T], F16, name=f"q_t{h}") for h in range(HPC)]
        self.k_t = [store.tile([128, T], F16, name=f"k_t{h}") for h in range(HPC)]
        self.v_sb = [store.tile([128, TT, 128], F16, name=f"v_sb{h}") for h in range(HPC)]
        self.ao_t = [store.tile([128, T], F16, name=f"ao_t{h}") for h in range(HPC)]

        self.xtp, self.qkfp, self.ropep, self.pp, self.rp, self.evp = (
            xtp, qkfp, ropep, pp, rp, evp)
        self.xt_tiles = {}

        def xt_dma(b, kc, g):
            # [128, 1024] tile (2KB lines) = 4 compute chunks of 256 tokens.
            # All 16 tiles of a group are live across those 4 chunks, so 32
            # bufs = current group + next group (cross-batch prefetch).
            t = xtp.tile([128, 1024], F16, tag="xt", bufs=32, name=f"xt_{b}_{kc}_{g}")
            nc.sync.dma_start(out=t, in_=xt[b, kc, :, ds(g * 1024, 1024)])
            self.xt_tiles[(b, kc, g)] = t

        # Startup DMA order = first-needed-first: the first qk matmul only
        # needs wqk pair 0 + the first half of xt[b0,kc<2,g0]. Batch-0 g0
        # tiles are filled by two half-DMAs so chunks 0-1 (tokens 0-511)
        # can start before the second halves arrive (subtile deps).
        halves = []
        for j in range(KC // 2):
            nc.sync.dma_start(out=self.wqk_sb[:, ds(2 * j, 2), :], in_=wqk[j])
            for kc in (2 * j, 2 * j + 1):
                t = self.xtp.tile([128, 1024], F16, tag="xt", bufs=32,
                                  name=f"xt_0_{kc}_0")
                nc.sync.dma_start(out=t[:, ds(0, 512)], in_=xt[0, kc, :, ds(0, 512)])
                self.xt_tiles[(0, kc, 0)] = t
                halves.append((t, kc))
        for j in range(KC // 4):
            nc.sync.dma_start(out=self.wv_sb[:, ds(4 * j, 4), :], in_=wv[j])
        for t, kc in halves:
            nc.sync.dma_start(out=t[:, ds(512, 512)], in_=xt[0, kc, :, ds(512, 512)])
        nc.sync.dma_start(out=self.cos_sb, in_=cos_in)
        nc.sync.dma_start(out=self.sin_sb, in_=sin_in)
        for kc in range(KC):
            xt_dma(0, kc, 1)
        nc.sync.dma_start(out=self.tri_sb, in_=tri_in)
        nc.sync.dma_start(out=self.ones_col, in_=ones_in)
        for h in range(HPC):
            nc.sync.dma_start(out=self.wproj_sb[:, h, :], in_=wproj[h])

        for b in range(B):
            self._stage_a(b)
            if b + 1 < B:
                # prefetch next batch; slot rotation (12 bufs) paces the queue
                for g in range(2):
                    for kc in range(KC):
                        xt_dma(b + 1, kc, g)
            self._stage_b(b)
            self._stage_c(b, y)

    # qkv projection + RoPE for batch b
    def _stage_a(self, b):
        nc, tc = self.nc, self.tc
        with tc.tile_pool(name=f"psA{b}", bufs=1, space="PSUM") as psA:
            for c in range(NACH):
                seg = ds(c * ACH, ACH)
                # qk phase: 4 head-col accumulators in 2 banks, ping-pong'd
                ps_a = psA.tile([128, 512], F32, tag="qka", bufs=2, name="ps_a")
                ps_b = psA.tile([128, 512], F32, tag="qkb", bufs=2, name="ps_b")
                ps_qk = [ps_a[:, ds(0, ACH)], ps_a[:, ds(ACH, ACH)],
                         ps_b[:, ds(0, ACH)], ps_b[:, ds(ACH, ACH)]]
                goff = (c % 4) * ACH
                for kc in range(KC):
                    xt_kc = self.xt_tiles[(b, kc, c // 4)][:, ds(goff, ACH)]
                    for m in range(4):
                        nc.tensor.matmul(
                            ps_qk[m], self.wqk_sb[:, kc, ds(m * 128, 128)], xt_kc,
                            start=(kc == 0 and m % 2 == 0),
                            stop=(kc == KC - 1 and m % 2 == 1))
                # v phase: single bank (evicted fast by ACT during next chunk)
                ps_v = psA.tile([128, 512], F32, tag="v", bufs=1, name="ps_v")
                for kc in range(KC):
                    xt_kc = self.xt_tiles[(b, kc, c // 4)][:, ds(goff, ACH)]
                    for t2 in range(2):
                        nc.tensor.matmul(
                            ps_v[:, ds(t2 * 256, 256)],
                            xt_kc[:, ds(t2 * 128, 128)], self.wv_sb[:, kc, :],
                            start=(kc == 0 and t2 == 0),
                            stop=(kc == KC - 1 and t2 == 1))
                    if c % 4 == 3:
                        del self.xt_tiles[(b, kc, c // 4)]
                # RoPE eviction: DVE casts psum->fp16, then fp16 ops at 4x
                for bank, ps in enumerate((ps_a, ps_b)):
                    qkf = self.qkfp.tile([128, 512], F16, tag="qkf", name="qkf")
                    nc.vector.tensor_copy(qkf, ps)
                    for half in range(2):
                        m = bank * 2 + half
                        h = m % 2
                        dst = (self.q_t if m < 2 else self.k_t)[h]
                        src = qkf[:, ds(half * ACH, ACH)]
                        sw = self.ropep.tile([128, ACH], F16, tag="sw", name="sw")
                        t1 = self.ropep.tile([128, ACH], F16, tag="t1", name="t1")
                        nc.vector.stream_shuffle(sw, src, mask=SWAP_MASK)
                        nc.vector.tensor_mul(t1, src, self.cos_sb[:, seg])
                        nc.vector.tensor_mul(sw, sw, self.sin_sb[:, seg])
                        nc.vector.tensor_add(dst[:, seg], t1, sw)
                # v eviction on ACT (Copy shares the exp_and_others table
                # with Exp, so no activation-table reloads; gpsimd cannot
                # read PSUM at all)
                for t2 in range(2):
                    for h in range(HPC):
                        nc.scalar.copy(self.v_sb[h][:, c * 2 + t2, :],
                                       ps_v[:, ds(t2 * 256, 256)][:, ds(h * 128, 128)])

    # causal attention for both heads (current batch): one flattened
    # software pipeline over (head, chunk, k-tile pair)
    def _stage_b(self, b):
        nc, tc = self.nc, self.tc

        def trim(i, nvalid):
            # diagonal k-tiles: restrict to valid q sub-range
            r = i - (nvalid - 4)
            return max(r, 0) * 128

        # work list: pairs of k-tiles. Head 0 walks chunks descending (the
        # first chunk after stage A is the longest -> deep pipeline from the
        # start); head 1 ascending (so ao chunk 0, which stage C consumes
        # first, is ready early). Within a chunk the 2 diagonal (trimmed)
        # pairs are interleaved in front of full pairs: their exp latency
        # then hides under the following full pair's matmul stream.
        pairs = []  # (u, jc, nvalid, i0, first, last)
        for u in range(HPC):
            for jc in ((3, 2, 1, 0) if u == 0 else (0, 1, 2, 3)):
                nvalid = (jc + 1) * 4
                np_ = nvalid // 2
                order = ([np_ - 2, np_ - 1] if np_ == 2 else
                         [np_ - 2, 0, np_ - 1] + list(range(1, np_ - 2)))
                for ei, p in enumerate(order):
                    pairs.append((u, jc, nvalid, 2 * p, ei == 0, ei == np_ - 1))
        # pool order matters: the stack allocator reuses stage A's banks in
        # creation order. psD/psO (first written ~4us into stage B) take the
        # qk banks whose RoPE evictions are still draining at the seam;
        # psS (written immediately) gets the early-freed v bank + virgin banks.
        with (
            tc.tile_pool(name=f"psD{b}", bufs=2, space="PSUM") as psD,
            tc.tile_pool(name=f"psO{b}", bufs=2, space="PSUM") as psO,
            tc.tile_pool(name=f"psS{b}", bufs=2, space="PSUM") as psS,
        ):
            inflight = {}
            ps_d = {}
            ps_o = {}
            for w in range(len(pairs) + LAGP):
                if w < len(pairs):
                    u, jc, nvalid, i0, first, last = pairs[w]
                    ps2 = psS.tile([128, 1024], F32, tag="s2", name="ps2")
                    pt2 = self.pp.tile([128, 1024], F16, tag="pt2", name="pt2")
                    los = [trim(i0, nvalid), trim(i0 + 1, nvalid)]
                    # member 0 right-aligned ([lo0:512]), member 1 LEFT-aligned
                    # at column 512: the pair region [lo0 : 1024-lo1] is then
                    # contiguous -> ONE exp per pair
                    offs = [los[0], 512]
                    for m, (lo, off) in enumerate(zip(los, offs)):
                        i = i0 + m
                        nc.tensor.matmul(
                            ps2[:, ds(off, 512 - lo)],
                            self.k_t[u][:, ds(i * 128, 128)],
                            self.q_t[u][:, ds(jc * QCH + lo, 512 - lo)],
                            start=True, stop=True)
                    reg = ds(los[0], 1024 - los[0] - los[1])
                    nc.scalar.activation(
                        pt2[:, reg], ps2[:, reg],
                        mybir.ActivationFunctionType.Exp, scale=INV_SQRT_D)
                    for m, (lo, off) in enumerate(zip(los, offs)):
                        if i0 + m >= nvalid - 4:  # diagonal: mask the triangle
                            mreg = ds(off, 128)
                            nc.vector.tensor_mul(pt2[:, mreg], pt2[:, mreg], self.tri_sb)
                    inflight[w] = (u, jc, nvalid, i0, first, last, los, pt2)
                j = w - LAGP
                if j >= 0 and j < len(pairs):
                    u, jc, nvalid, i0, first, last, los, pt2 = inflight.pop(j)
                    key = (u, jc)
                    if first:
                        ps_d[key] = psD.tile([1, 512], F32, tag="d", name="ps_d")
                        ps_o[key] = psO.tile([128, 512], F32, tag="o", name="ps_o")
                    # den,den then O,O: consecutive denominators share the
                    # ones lhsT so every ld_weights hides under the previous
                    # matmul's stream (den,O,den,O thrashes the shadow load)
                    for m, lo in enumerate(los):
                        src = pt2[:, ds(lo if m == 0 else 512, 512 - lo)]
                        nc.tensor.matmul(
                            ps_d[key][:, ds(lo, 512 - lo)], self.ones_col, src,
                            start=(first and m == 0), stop=(last and m == 1))
                    for m, lo in enumerate(los):
                        i = i0 + m
                        src = pt2[:, ds(lo if m == 0 else 512, 512 - lo)]
                        nc.tensor.matmul(
                            ps_o[key][:, ds(lo, 512 - lo)], self.v_sb[u][:, i, :], src,
                            start=(first and m == 0), stop=(last and m == 1))
                    if last:  # chunk complete: normalize
                        r_sb = self.rp.tile([1, QCH], F32, tag="r", name="r_sb")
                        nc.vector.reciprocal_approx_fast(out=r_sb, in_=ps_d.pop(key))
                        rbc = self.rp.tile([128, QCH], F32, tag="rbc", name="rbc")
                        nc.gpsimd.partition_broadcast(rbc, r_sb)
                        nc.vector.tensor_mul(
                            self.ao_t[u][:, ds(jc * QCH, QCH)], ps_o.pop(key), rbc)

    # out-projection partial for batch b (fp16 partial written to HBM)
    def _stage_c(self, b, y):
        nc, tc = self.nc, self.tc
        with tc.tile_pool(name=f"psY{b}", bufs=3, space="PSUM") as psY:
            for tt in range(TT):
                # one [128,2048] staging tile per token-tile: 4KB DMA lines
                # and a single writeback trigger (on the ACT hardware-DGE
                # queue; the sync queue is busy pacing the x prefetch and a
                # gpsimd software-DGE trigger costs ~0.64us)
                yv = self.evp.tile([128, 2048], F16, tag="yv", bufs=4, name="yv")
                for nck in range(C // 512):
                    ps_y = psY.tile([128, 512], F32, tag="y", name="ps_y")
                    for h in range(HPC):
                        nc.tensor.matmul(
                            ps_y, self.ao_t[h][:, ds(tt * 128, 128)],
                            self.wproj_sb[:, h, ds(nck * 512, 512)],
                            start=(h == 0), stop=(h == HPC - 1))
                    # alternate eviction engine: neither ACT nor DVE alone
                    # keeps pace with the PE here (gpsimd cannot read PSUM)
                    if nck % 2 == 0:
                        nc.scalar.copy(yv[:, ds(nck * 512, 512)], ps_y)
                    else:
                        nc.vector.tensor_copy(yv[:, ds(nck * 512, 512)], ps_y)
                    if b == B - 1 and tt >= TT - 2:
                        # end of kernel: stream each quarter out as soon as
                        # it's evicted to shorten the final DMA drain
                        nc.sync.dma_start(out=y[b, tt, :, ds(nck * 512, 512)],
                                          in_=yv[:, ds(nck * 512, 512)])
                if not (b == B - 1 and tt >= TT - 2):
                    # sync queue: empty during B/C (the b+1 x prefetch fully
                    # unblocks during stage A), so no head-of-line risk
                    nc.sync.dma_start(out=y[b, tt, :, :], in_=yv)


def prep_inputs(x, w_qkv, w_proj):
    """Host-side sharding: returns the per-core input maps."""
    x = np.asarray(x, dtype=np.float32)
    w_qkv = np.asarray(w_qkv, dtype=np.float32)
    w_proj = np.asarray(w_proj, dtype=np.float32)

    # x^T per batch: [B, C, T] -> tiled [B, KC, 128, T], fp16
    xt = np.ascontiguousarray(x.transpose(0, 2, 1)).reshape(B, KC, 128, T).astype(np.float16)

    # RoPE tables (fp32 math, fp16 storage), d-permuted + sign-folded
    inv_freq = (1.0 / (10000.0 ** (np.arange(0, D, 2, dtype=np.float32) / D))).astype(np.float32)
    t = np.arange(T, dtype=np.float32)
    freqs = np.einsum("i,j->ij", t, inv_freq).astype(np.float32)  # [T, 64]
    emb = np.concatenate([freqs, freqs], axis=-1)  # [T, 128]
    cos_full = np.cos(emb).astype(np.float32)  # [T, 128]
    sin_full = np.sin(emb).astype(np.float32)
    sgn = np.where(np.arange(D) < D // 2, np.float32(-1.0), np.float32(1.0))
    cos_t = np.ascontiguousarray(cos_full[:, PERM].T).astype(np.float16)  # [128, T]
    sin_t = np.ascontiguousarray((sin_full * sgn)[:, PERM].T).astype(np.float16)

    # triangular mask for the 128-wide diagonal sub-tiles
    kp = np.arange(128)[:, None]
    qf = np.arange(128)[None, :]
    tri = (qf >= kp).astype(np.float16)

    in_maps = []
    for g in range(NCORES):
        heads = [HPC * g + h for h in range(HPC)]
        # wqk: [C, 512] cols = [q_h0, q_h1, k_h0, k_h1], d-permuted
        cols = []
        for base in (0, C):  # q block, k block
            for hh in heads:
                cols.append(w_qkv[:, base + hh * 128 + PERM])
        # packed so each DMA row is >=2KB: kc-pairs for wqk, kc-quads for wv
        wqk_g = np.ascontiguousarray(np.concatenate(cols, axis=1)).reshape(
            KC // 2, 2, 128, 512).transpose(0, 2, 1, 3).reshape(KC // 2, 128, 1024)
        wqk_g = np.ascontiguousarray(wqk_g).astype(np.float16)
        wv_g = np.ascontiguousarray(
            np.concatenate([w_qkv[:, 2 * C + hh * 128:2 * C + (hh + 1) * 128] for hh in heads], axis=1)
        ).reshape(KC // 4, 4, 128, 256).transpose(0, 2, 1, 3).reshape(KC // 4, 128, 1024)
        wv_g = np.ascontiguousarray(wv_g).astype(np.float16)
        wproj_g = np.ascontiguousarray(
            np.stack([w_proj[hh * 128:(hh + 1) * 128, :] for hh in heads])
        ).astype(np.float16)
        in_maps.append({
            "xt": xt,
            "wqk": wqk_g,
            "wv": wv_g,
            "wproj": wproj_g,
            "cos_t": cos_t,
            "sin_t": sin_t,
            "tri": tri,
            "ones": np.ones((128, 1), dtype=np.float16),
        })
    return in_maps


_NC_CACHE = {}


def get_program():
    if "nc" not in _NC_CACHE:
        _NC_CACHE["nc"] = build_program()
    return _NC_CACHE["nc"]


def kernel(x, w_qkv, w_proj, b_proj):
    from concourse import bass_utils

    nc = get_program()
    in_maps = prep_inputs(x, w_qkv, w_proj)
    res = bass_utils.run_bass_kernel_spmd(nc, in_maps, core_ids=list(range(NCORES)))
    acc = None
    for r in res.results:
        part = r["y"].astype(np.float32).reshape(B, T, C)
        acc = part if acc is None else acc + part
    return (acc + np.asarray(b_proj, dtype=np.float32)).astype(np.float32)
